# revision 13
# baseline (speedup 1.0000x reference)
"""Trainium2 Bass kernel for the GPCwSTU rollout (nn_GPCwSTU_72576357368005).

Math restructure: the sequential rollout is the lower-triangular linear system
    u_t = d_t - sum_{s<t} K (Ecat^T (phi_s (x) u_s)),
with d_t = bias + sum_i E[:,:,i] @ w_{t-4+i} precomputable in parallel.
The coupling is weak, so Richardson iteration with EXACT cross-core offsets
    u <- d - prefix_t( K Ecat^T (phi (x) u) )
reaches 1.5e-3 (NITERS=2) / 2.7e-3 (NITERS=1) loss error (validated in
fp64/quantized numpy emulation against the sequential reference; gate 2e-2).

Implementation notes:
  - two-step contraction y = K (Ecat^T O) instead of y = F^T O: kills the
    137us replicated Fmat=Ecat@K^T precompute of the 720us baseline.
  - fp8 (e4m3, DoubleRow) for the big Ecat^T O contraction: 2x tensor rate,
    and the 10.5MB scaled Ecat stays resident in SBUF.
  - the kernel carries v = -u: the prefix scan emits v directly via
    v_t = (y_{t-1} + state) + (d_{t-1} - d_t), killing the u = d - a
    subtract. All loss terms are quadratic, so the sign cancels.
  - the final X-offset AllGather overlaps with Q*Xlocal / R*v matmuls; the
    offset enters as per-partition-scalar fixups afterwards. PSUM banks are
    reused across phases via tile views (8-bank budget).
  - input DMA posts are spread over the sync/scalar/gpsimd/vector queues so
    no engine queue stalls compute or the collectives.

Scale bookkeeping: Ecat is scaled by S_E and phi by S_O for fp8 range; z / X
on device are scaled by S_E*S_O. K^T is pre-divided by S_E*S_O (y, v true
scale), Q by (S_E*S_O)^2 (X*(QX) true scale).

Layouts are feature-major ([feature, t]); t is sharded 256 steps/core.
"""

import sys

sys.path.insert(0, "/opt/trn_rl_repo")

import numpy as np
import ml_dtypes

import concourse.bass as bass
import concourse.bacc as bacc
import concourse.mybir as mybir
from concourse import tile
from concourse.bass_utils import run_bass_kernel_spmd

BF16 = mybir.dt.bfloat16
F32 = mybir.dt.float32
FP8 = mybir.dt.float8e4
AL = mybir.AluOpType
ACT = mybir.ActivationFunctionType
DR = mybir.MatmulPerfMode.DoubleRow

T, N, MC, KF, M = 2048, 1024, 512, 20, 5
NCORES = 8
TS = T // NCORES          # 256 timesteps per core
NK = N // 128             # 8 tiles over state dim
CT = MC // 128            # 4 tiles over control dim
ICT = (KF * MC) // 128    # 80 tiles over the (filter, control) contraction
NITERS = 2                # exact-offset Richardson iterations

S_E = 4096.0              # fp8 scale on Ecat
S_O = 64.0                # fp8 scale on O = phi (x) u  (folded into phiB)

_CACHE = {}


def build_nc(debug=False, reps=1):
    nc = bacc.Bacc(None, target_bir_lowering=False, debug=False)

    # ---- I/O ----
    wT_d = nc.declare_dram_parameter("wT", [N, TS + M - 1], BF16, isOutput=False)
    ET_d = nc.declare_dram_parameter("ET", [M, N, MC], BF16, isOutput=False)
    Ecat8_d = nc.declare_dram_parameter("Ecat8", [KF * MC, N], FP8, isOutput=False)
    KT_d = nc.declare_dram_parameter("KT", [N, MC], BF16, isOutput=False)
    Q_d = nc.declare_dram_parameter("Q", [N, N], BF16, isOutput=False)
    R_d = nc.declare_dram_parameter("R", [MC, MC], BF16, isOutput=False)
    phiB_d = nc.declare_dram_parameter("phiB", [128, KF, TS], BF16, isOutput=False)
    biasT_d = nc.declare_dram_parameter("biasT", [MC, 1], F32, isOutput=False)
    biasTn_d = nc.declare_dram_parameter("biasTn", [MC, 1], F32, isOutput=False)
    mask_d = nc.declare_dram_parameter("mask", [NCORES, 1], F32, isOutput=False)
    loss_d = nc.declare_dram_parameter("loss", [1, TS], F32, isOutput=True)
    if debug:
        dbg_v = nc.declare_dram_parameter("dbg_v", [128, CT, TS], F32, isOutput=True)
        dbg_z = nc.declare_dram_parameter("dbg_z", [128, NK, TS], F32, isOutput=True)
        dbg_X = nc.declare_dram_parameter("dbg_X", [128, NK, TS], F32, isOutput=True)

    # collective bounce buffers
    bsum_d = nc.dram_tensor("bsum", [MC], F32)
    bgat_d = nc.dram_tensor("bgat", [NCORES, MC], F32, addr_space="Shared")
    bxsum_d = nc.dram_tensor("bxsum", [N], F32)
    bxgat_d = nc.dram_tensor("bxgat", [NCORES, N], F32, addr_space="Shared")

    with tile.TileContext(nc) as tc:
        with (
            tc.tile_pool(name="const", bufs=1) as cpool,
            tc.tile_pool(name="live", bufs=1) as opool,
            tc.tile_pool(name="work", bufs=2) as wpool,
        ):
            # ---- constants; queues chosen to avoid stalling compute ----
            phiB = cpool.tile([128, KF, TS], BF16)
            KTs = cpool.tile([128, NK, MC], BF16)
            nc.gpsimd.dma_start(KTs[:], KT_d.ap().rearrange("(k p) c -> p k c", p=128))
            Rs = cpool.tile([128, CT, MC], BF16)
            nc.gpsimd.dma_start(Rs[:], R_d.ap().rearrange("(k p) c -> p k c", p=128))
            biasT = cpool.tile([128, CT, 1], F32)
            nc.gpsimd.dma_start(biasT[:], biasT_d.ap().rearrange("(c p) one -> p c one", p=128))
            biasTn = cpool.tile([128, CT, 1], F32)
            nc.gpsimd.dma_start(biasTn[:], biasTn_d.ap().rearrange("(c p) one -> p c one", p=128))
            mask = cpool.tile([NCORES, 1], F32)
            nc.gpsimd.dma_start(mask[:], mask_d[:])
            zeros = cpool.tile([128, TS], F32)
            nc.vector.memset(zeros[:], 0.0)
            ones = cpool.tile([128, 1], F32)
            nc.vector.memset(ones[:], 1.0)

            for rep in range(reps):
                # ---- long-lived state ----
                vbf = opool.tile([128, CT, TS], BF16)   # v = (-1)^k u (bf16)
                dd = opool.tile([128, CT, TS], BF16)    # dd[t] = d[t-1] - d[t]
                ddn = opool.tile([128, CT, TS], BF16)   # -dd
                d0n = opool.tile([128, CT, 1], F32)     # -(d[0]) per ct
                d0p = opool.tile([128, CT, 1], F32)     # +(d[0]) per ct
                O8 = opool.tile([128, KF, CT, TS], FP8)
                zsb = opool.tile([128, NK, TS], BF16)
                ysb = opool.tile([128, CT, TS], F32)
                Ecat8 = opool.tile([128, ICT, N], FP8)
                Qs = opool.tile([128, NK, N], BF16)
                Xbf = opool.tile([128, NK, TS], BF16)
                Xp = opool.tile([128, NK, TS], F32)
                offS = opool.tile([128, CT, 1], F32)
                Bloc = opool.tile([128, CT, 1], F32)

                # ---- phase 1: dps = sum_i E_i @ w_shift_i (bf16 matmuls) ----
                with (
                    tc.tile_pool(name="p1", bufs=1) as p1,
                    tc.tile_pool(name="p1s", bufs=2) as p1s,
                    tc.tile_pool(name="p1ps", bufs=1, space="PSUM") as p1ps,
                ):
                    wTs = p1.tile([128, NK, TS + M - 1], BF16)
                    nc.scalar.dma_start(wTs[:], wT_d.ap().rearrange("(k p) t -> p k t", p=128))
                    dps = p1ps.tile([128, CT, TS], F32)
                    et_tiles = []
                    for i in range(M):
                        ETs = p1s.tile([128, NK, MC], BF16, tag="et", bufs=3)
                        eng = nc.sync if i % 2 == 0 else nc.scalar
                        eng.dma_start(ETs[:], ET_d[i].rearrange("(k p) c -> p k c", p=128))
                        et_tiles.append(ETs)
                    for i in range(M):
                        ETs = et_tiles[i]
                        for k in range(NK):
                            for ct in range(CT):
                                nc.tensor.matmul(
                                    dps[:, ct, :],
                                    ETs[:, k, ct * 128:(ct + 1) * 128],
                                    wTs[:, k, i:i + TS],
                                    start=(i == 0 and k == 0 and ct % 2 == 0),
                                    stop=(i == M - 1 and k == NK - 1 and ct % 2 == 1),
                                )
                    nc.scalar.dma_start(phiB[:], phiB_d[:])
                    # resident fp8 Ecat, split over scalar/gpsimd queues
                    for kf in range(KF):
                        eng = nc.scalar if kf < 7 else nc.gpsimd
                        eng.dma_start(
                            Ecat8[:, kf * CT:(kf + 1) * CT, :],
                            Ecat8_d[kf * MC:(kf + 1) * MC, :].rearrange("(k p) n -> p k n", p=128),
                        )
                    # u0 = d (carried negated later); dd for the fused scans
                    for ct in range(CT):
                        nc.scalar.activation(vbf[:, ct, :], dps[:, ct, :], ACT.Identity,
                                             bias=biasT[:, ct, :], scale=1.0)
                        nc.scalar.activation(d0n[:, ct, :], dps[:, ct, 0:1], ACT.Identity,
                                             bias=biasTn[:, ct, :], scale=-1.0)
                        nc.scalar.activation(d0p[:, ct, :], dps[:, ct, 0:1], ACT.Identity,
                                             bias=biasT[:, ct, :], scale=1.0)

                # deferred big weights (needed only in the loss phase)
                nc.sync.dma_start(Qs[:], Q_d.ap().rearrange("(k p) n -> p k n", p=128))

                with (
                    tc.tile_pool(name="zps_p", bufs=1, space="PSUM") as zps_p,
                    tc.tile_pool(name="yps_p", bufs=1, space="PSUM") as yps_p,
                    tc.tile_pool(name="off_p", bufs=1, space="PSUM") as off_p,
                    tc.tile_pool(name="sm_p", bufs=1, space="PSUM") as sm_p,
                ):
                    zps = zps_p.tile([128, NK, TS], F32)
                    yps = yps_p.tile([128, CT, TS], F32)
                    # offp bank also hosts offX (cols CT..CT+NK) and
                    # qoffX (cols CT+NK..CT+2NK) in the loss phase
                    offp = off_p.tile([128, CT + 2 * NK, 1], F32)
                    lps_t = sm_p.tile([1, TS], F32)

                    for it in range(NITERS + 1):
                        last = it == NITERS
                        # O = phi (x) v, per-kf chunks pipelined into matmuls.
                        # On pass 0, v = +d; the global sign flips once at
                        # iteration 1 and stays (quadratic loss is sign-blind).
                        for kf in range(KF):
                            nc.vector.tensor_tensor(
                                O8[:, kf, :, :], vbf[:, :, :],
                                phiB[:, kf, :].unsqueeze(1).broadcast_to([128, CT, TS]),
                                op=AL.mult,
                            )
                            for h in range(2):
                                kk = kf * CT + h * 2
                                for nt in range(NK):
                                    nc.tensor.matmul(
                                        zps[:, nt, :],
                                        Ecat8[:, kk:kk + 2, nt * 128:(nt + 1) * 128],
                                        O8[:, kf, h * 2:h * 2 + 2, :],
                                        start=(kf == 0 and h == 0 and nt % 2 == 0),
                                        stop=(kf == KF - 1 and h == 1 and nt % 2 == 1),
                                        perf_mode=DR,
                                    )
                        if it == 0:
                            nc.vector.tensor_tensor(dd[:, :, 1:TS], vbf[:, :, 0:TS - 1],
                                                    vbf[:, :, 1:TS], op=AL.subtract)
                            nc.vector.tensor_tensor(ddn[:, :, 1:TS], vbf[:, :, 1:TS],
                                                    vbf[:, :, 0:TS - 1], op=AL.subtract)
                        if not last:
                            # y = K z ; block sums ; AllGather ; fused v-scan
                            for nt in range(NK):
                                nc.scalar.copy(zsb[:, nt, :], zps[:, nt, :])
                            for ct in range(CT):
                                for k in range(NK):
                                    nc.tensor.matmul(
                                        yps[:, ct, :],
                                        KTs[:, k, ct * 128:(ct + 1) * 128],
                                        zsb[:, k, :],
                                        start=(k == 0 and ct % 2 == 0),
                                        stop=(k == NK - 1 and ct % 2 == 1),
                                    )
                            for ct in range(CT):
                                nc.scalar.activation(ysb[:, ct, :], yps[:, ct, :], ACT.Identity,
                                                     bias=0.0, scale=1.0,
                                                     accum_out=Bloc[:, ct, :])
                            nc.sync.dma_start(bsum_d.ap().rearrange("(c p) -> p c", p=128),
                                              Bloc[:, :, 0])
                            nc.gpsimd.collective_compute(
                                "AllGather", AL.bypass,
                                ins=[bsum_d[:]], outs=[bgat_d[:]],
                                replica_groups=[list(range(NCORES))],
                            )
                            gat = wpool.tile([NCORES, MC], F32, tag="gat")
                            nc.gpsimd.dma_start(gat[:], bgat_d[:])
                            for ct in range(CT):
                                nc.tensor.matmul(
                                    offp[:, ct, :], gat[:, ct * 128:(ct + 1) * 128], mask[:],
                                    start=(ct == 0), stop=(ct == CT - 1),
                                )
                            # v_{k+1} = prefix(y'_k) + off'_k + (-1)^{k+1} d
                            # emitted directly by the scan:
                            #   state_t = (y'_{t-1} + state) + (-1)^k dd_t
                            # with initial = off' + (-1)^{k+1} d_0.
                            ddk = dd if it % 2 == 0 else ddn
                            d0k = d0n if it % 2 == 0 else d0p
                            for ct in range(CT):
                                nc.scalar.activation(offS[:, ct, :], offp[:, ct, :], ACT.Identity,
                                                     bias=d0k[:, ct, :], scale=1.0)
                                nc.scalar.activation(vbf[:, ct, 0:1], offp[:, ct, :], ACT.Identity,
                                                     bias=d0k[:, ct, :], scale=1.0)
                            for ct in range(CT):
                                nc.vector.tensor_tensor_scan(
                                    vbf[:, ct, 1:TS], ysb[:, ct, 0:TS - 1], ddk[:, ct, 1:TS],
                                    offS[:, ct, :], op0=AL.add, op1=AL.add,
                                )
                            if debug and rep == 0 and it == NITERS - 1:
                                ud = wpool.tile([128, CT, TS], F32, tag="ud")
                                for ct in range(CT):
                                    nc.vector.tensor_copy(ud[:, ct, :], vbf[:, ct, :])
                                nc.sync.dma_start(dbg_v[:], ud[:])
                        else:
                            # final pass: X offsets via AllGather of z col sums
                            BX = wpool.tile([128, NK, 1], F32, tag="bx")
                            for nt in range(NK):
                                nc.scalar.activation(Xp[:, nt, :], zps[:, nt, :], ACT.Identity,
                                                     bias=0.0, scale=1.0,
                                                     accum_out=BX[:, nt, :])
                            nc.sync.dma_start(bxsum_d.ap().rearrange("(c p) -> p c", p=128),
                                              BX[:, :, 0])
                            nc.gpsimd.collective_compute(
                                "AllGather", AL.bypass,
                                ins=[bxsum_d[:]], outs=[bxgat_d[:]],
                                replica_groups=[list(range(NCORES))],
                            )
                            gatx = wpool.tile([NCORES, N], F32, tag="gatx")
                            nc.gpsimd.dma_start(gatx[:], bxgat_d[:])
                            # local X prefix (zero offset) while the AllGather flies
                            for nt in range(NK):
                                nc.vector.memset(Xbf[:, nt, 0:1], 0.0)
                                nc.vector.tensor_tensor_scan(
                                    Xbf[:, nt, 1:TS], Xp[:, nt, 0:TS - 1], zeros[:, 0:TS - 1],
                                    0.0, op0=AL.add, op1=AL.add,
                                )
                            if debug and rep == 0:
                                nc.sync.dma_start(dbg_z[:], Xp[:])

                    # ---- loss phase; PSUM banks reused via views ----
                    # qxps := zps banks, rups := yps banks,
                    # offX / qoffX / lps in the `smalls` spare bank.
                    qxps = zps
                    rups = yps
                    with tc.tile_pool(name="p5", bufs=1) as p5:
                        offxS = p5.tile([128, NK, 1], F32)
                        offxB = p5.tile([128, NK, 1], BF16)
                        qoffS = p5.tile([128, NK, 1], F32)
                        # R v and Q Xlocal run during the X-AllGather
                        for ct in range(CT):
                            for k in range(CT):
                                nc.tensor.matmul(
                                    rups[:, ct, :], Rs[:, k, ct * 128:(ct + 1) * 128],
                                    vbf[:, k, :],
                                    start=(k == 0 and ct % 2 == 0),
                                    stop=(k == CT - 1 and ct % 2 == 1),
                                )
                        for nt in range(NK):
                            for k in range(NK):
                                nc.tensor.matmul(
                                    qxps[:, nt, :], Qs[:, k, nt * 128:(nt + 1) * 128],
                                    Xbf[:, k, :],
                                    start=(k == 0 and nt % 2 == 0),
                                    stop=(k == NK - 1 and nt % 2 == 1),
                                )
                        # offX from the gathered sums (own open/close cycle)
                        for nt in range(NK):
                            nc.tensor.matmul(
                                offp[:, CT + nt, :], gatx[:, nt * 128:(nt + 1) * 128], mask[:],
                                start=(nt == 0), stop=(nt == NK - 1),
                            )
                        for nt in range(NK):
                            nc.scalar.copy(offxS[:, nt, :], offp[:, CT + nt, :])
                            nc.scalar.copy(offxB[:, nt, :], offp[:, CT + nt, :])
                        # qoff = Q offX (reopen the offp bank)
                        for nt in range(NK):
                            for k in range(NK):
                                nc.tensor.matmul(
                                    offp[:, CT + NK + nt, :],
                                    Qs[:, k, nt * 128:(nt + 1) * 128], offxB[:, k, :],
                                    start=(nt == 0 and k == 0),
                                    stop=(nt == NK - 1 and k == NK - 1),
                                )
                        for nt in range(NK):
                            nc.scalar.copy(qoffS[:, nt, :], offp[:, CT + NK + nt, :])
                        # true X = Xlocal + offX ; prod = X * (QXlocal + qoff)
                        prod = p5.tile([128, NK, TS], F32)
                        prodr = p5.tile([128, CT, TS], F32)
                        for nt in range(NK):
                            nc.vector.tensor_scalar_add(Xbf[:, nt, :], Xbf[:, nt, :],
                                                        offxS[:, nt, :])
                        if debug and rep == 0:
                            xd = wpool.tile([128, NK, TS], F32, tag="xd")
                            for nt in range(NK):
                                nc.vector.tensor_copy(xd[:, nt, :], Xbf[:, nt, :])
                            nc.sync.dma_start(dbg_X[:], xd[:])
                        for ct in range(CT):
                            nc.vector.tensor_tensor(prodr[:, ct, :], vbf[:, ct, :],
                                                    rups[:, ct, :], op=AL.mult)
                        # the R-side loss partial accumulates during the AG
                        lps = lps_t[:]
                        for ct in range(CT):
                            nc.tensor.matmul(lps, ones[:], prodr[:, ct, :],
                                             start=(ct == 0), stop=False)
                        for nt in range(NK):
                            nc.vector.scalar_tensor_tensor(
                                prod[:, nt, :], qxps[:, nt, :], qoffS[:, nt, :], Xbf[:, nt, :],
                                op0=AL.add, op1=AL.mult,
                            )
                        for nt in range(NK):
                            nc.tensor.matmul(lps, ones[:], prod[:, nt, :],
                                             start=False, stop=(nt == NK - 1))
                        loss = wpool.tile([1, TS], F32, tag="loss")
                        nc.vector.tensor_copy(loss[:], lps)
                        nc.sync.dma_start(loss_d[:], loss[:])

    nc.compile()
    return nc


def _prep_inputs(inputs):
    f32 = np.float32
    bf = ml_dtypes.bfloat16
    f8 = ml_dtypes.float8_e4m3
    E = np.asarray(inputs["E"], f32)            # [MC, N, M]
    K = np.asarray(inputs["K"], f32)            # [MC, N]
    E_stu = np.asarray(inputs["E_stu"], f32)    # [KF, MC, N]
    phi = np.asarray(inputs["phi"], f32)        # [T, KF]
    w = np.asarray(inputs["w_test"], f32)       # [T, N]
    Q = np.asarray(inputs["Q"], f32)
    R = np.asarray(inputs["R"], f32)
    bias = np.asarray(inputs["bias"], f32)

    ET = np.ascontiguousarray(E.transpose(2, 1, 0)).astype(bf)   # [M, N, MC]
    Ecat = E_stu.reshape(KF * MC, N)
    Ecat8 = np.clip(Ecat * S_E, -240, 240).astype(f8)
    KTb = np.ascontiguousarray(K.T / (S_E * S_O)).astype(bf)
    Qb = (Q / (S_E * S_O) ** 2).astype(bf)
    Rb = R.astype(bf)
    biasT = np.ascontiguousarray(bias[:, None]).astype(f32)
    # w^T padded with M-1 zero columns at the left (for t<0 history)
    wTp = np.concatenate([np.zeros((N, M - 1), f32), np.ascontiguousarray(w.T)], axis=1)
    phiT = np.ascontiguousarray(phi.T) * S_O                      # [KF, T]

    in_maps = []
    for r in range(NCORES):
        t0 = r * TS
        wT_r = np.ascontiguousarray(wTp[:, t0:t0 + TS + M - 1]).astype(bf)
        phiB_r = np.broadcast_to(
            phiT[None, :, t0:t0 + TS], (128, KF, TS)
        ).astype(bf)
        mask_r = np.zeros((NCORES, 1), f32)
        mask_r[:r] = 1.0
        in_maps.append({
            "wT": wT_r, "ET": ET, "Ecat8": Ecat8, "KT": KTb,
            "Q": Qb, "R": Rb, "phiB": np.ascontiguousarray(phiB_r),
            "biasT": biasT, "biasTn": -biasT, "mask": mask_r,
        })
    return in_maps


def kernel(**inputs) -> np.ndarray:
    if "nc" not in _CACHE:
        _CACHE["nc"] = build_nc()
    nc = _CACHE["nc"]
    in_maps = _prep_inputs(inputs)
    res = run_bass_kernel_spmd(nc, in_maps, list(range(NCORES)))
    out = np.concatenate([res.results[r]["loss"][0] for r in range(NCORES)])
    return out.astype(np.float32)
